# revision 2
# baseline (speedup 1.0000x reference)
"""Trainium2 Bass kernel for nn_MoEFFN_86895778333203 — v2.

Dense MoE FFN: B=4, S=2048, D=512, F=2048, E=8 routed experts + 1 shared
expert, gating-weighted combine.

Sharding v2: 2D (token-group x F-half). The 8 cores form a 4x2 grid:
core = 2*group + half. Each core owns 2048 tokens (group) and 1024
d_ff columns (half) of every expert's W1/W2 (+ the shared expert folded
in as expert #8). ALL weights for a core fit in SBUF (18.9 MB bf16), so
after the prologue there is ZERO steady-state DMA — the rep loop is pure
compute. The two halves' partial outputs (contraction over f splits
linearly; gelu is elementwise in f) are summed pairwise on the host,
which also folds in the b2/bs2 bias image.

Per-core device algorithm (all matmuls bf16, fp32 PSUM accumulation):
  hT_e = gelu_tanh(W1h_e^T x^T + b1h_e)      # [Fh,Tc] layout, f on parts
  y_e  = hT_e^T W2h_e                        # [Tc,D], t on partitions
  acc  = sum_e w_e * y_e                     # per-token combine weights
with w_e = (1-sg)*g_e routed, w_8 = sg shared. acc is bf16 in SBUF
(16 KB/partition); host re-sums halves in fp32.

Scheduling notes: GEMM1 processes token-chunk PAIRS sharing the
stationary W1 tile (the second LDWEIGHTS is elided at BIR level) with
the two accumulations interleaved across two PSUM banks; GEMM2
interleaves t-tile pairs across two banks the same way (the baseline's
single-bank runs of 16 were measurably slower).
"""

import numpy as np
import ml_dtypes

import concourse.bass as bass
import concourse.tile as tile
from concourse import mybir

# ---------------------------------------------------------------------------
# Patch TileContext._drain_and_barrier: the stock version attaches one sem
# wait per live logical proc to a single Drain instruction; this walrus
# build caps sync-wait commands per instruction, so split the waits across
# several drains (each observes <=CHUNK procs; same-engine program order
# makes the union equivalent).
# ---------------------------------------------------------------------------
from concourse.vector_clock import ScopedClock, VectorClock

_DRAIN_CHUNK = 4


def _split_drain_and_barrier(self, tick_clock, wait_clock):
    gc = tick_clock.global_clock
    n = len(gc)
    for s in range(0, n, _DRAIN_CHUNK):
        vec = [0] * n
        nonzero = False
        for i in range(s, min(s + _DRAIN_CHUNK, n)):
            vec[i] = gc[i]
            nonzero = nonzero or gc[i] > 0
        if not nonzero:
            continue
        drain_inst = self.nc.sync.drain()
        wait_clock.add_sem_waits(drain_inst.ins, ScopedClock({None: VectorClock(vec)}))
    self.nc.all_engine_barrier()
    assert self.sems is not None
    popped = self.nc._tile_sem_poison_stack.pop()
    assert popped is self._sem_poison
    self.nc.clear_and_free_semaphores(list(self.sems.allocated().values()))
    self.nc.all_engine_barrier()


tile.TileContext._drain_and_barrier = _split_drain_and_barrier


def _split_excess_waits(nc):
    """This walrus build allows at most 1 sync-wait command per instruction
    (2 for EventSemaphore ops). Tile attaches up to ~4. Hoist the excess
    onto standalone EventSemaphore wait instructions inserted immediately
    before the owner in the same block (same engine => program order is
    preserved, semantics identical)."""
    uid = 0
    for fn in nc.m.functions:
        for bb in fn.blocks:
            il = bb.instructions
            i = 0
            while i < len(il):
                inst = il[i]
                si = inst.sync_info
                waits = list(si.on_wait) if si and si.on_wait else []
                cap = 2 if isinstance(inst, mybir.InstEventSemaphore) else 1
                if len(waits) > cap:
                    keep = waits[-cap:]
                    extra = waits[:-cap]
                    new_insts = []
                    for j in range(0, len(extra), 2):
                        uid += 1
                        new_insts.append(
                            mybir.InstEventSemaphore(
                                name=f"bass_splitwait_{uid}",
                                engine=inst.engine,
                                sync_info=mybir.SyncInfo(
                                    on_wait=list(extra[j : j + 2]), on_update=[]
                                ),
                            )
                        )
                    si.on_wait = keep
                    for k, wi in enumerate(new_insts):
                        il.insert(i + k, wi)
                    i += len(new_insts)
                i += 1


def _elide_redundant_ldweights(nc):
    """Drop an InstLdweights that reloads the exact stationary operand the PE
    already holds (same memref/offset/ap/dtype, only matmuls in between, no
    sync attached). walrus's own --enable-ldw-opt pass does this but crashes
    on this toolchain, so do it on the BIR directly: a matmul with no fresh
    LDWEIGHTS keeps using the current foreground weights."""
    for fn in nc.m.functions:
        for bb in fn.blocks:
            il = bb.instructions
            keep = []
            last_key = None
            for inst in il:
                nm = type(inst).__name__
                if nm == "InstLdweights":
                    ap = inst.ins[0]
                    key = (ap.memref, ap.offset, str(ap.ap), str(ap.dtype))
                    si = inst.sync_info
                    clean = not (si and (si.on_wait or si.on_update))
                    if key == last_key and clean:
                        continue  # elide
                    last_key = key
                elif nm != "InstMatmult" and str(inst.engine).endswith("PE"):
                    last_key = None
                keep.append(inst)
            if len(keep) != len(il):
                il[:] = keep


# ---------------------------------------------------------------------------
# Problem shapes (hardcoded per contract)
# ---------------------------------------------------------------------------
B, S, D, F, E = 4, 2048, 512, 2048, 8
NCORES = 8
NGROUP = 4                # token groups
NHALF = 2                 # F halves
NTOK = B * S              # 8192 tokens total
T = NTOK // NGROUP        # 2048 tokens per core (token group)
FH = F // NHALF           # 1024 d_ff columns per core
NE = E + 1                # 8 routed + shared
P = 128
DT = D // P               # 4 k-tiles for GEMM1
FTH = FH // P             # 8 f-tiles per half (GEMM1 m-tiles / GEMM2 k-tiles)
TCHUNK = 512              # GEMM1 rhs free-dim chunk
NTC = T // TCHUNK         # 4 chunks
TSUB = TCHUNK // P        # 4 t-tiles of 128 per chunk
NTT = T // P              # 16 t-tiles per core

BF16 = mybir.dt.bfloat16
F32 = mybir.dt.float32
GELU = mybir.ActivationFunctionType.Gelu_apprx_tanh


def _build_program(reps: int = 1):
    nc = bass.Bass()
    xT = nc.declare_dram_parameter("xT", [D, T], BF16, isOutput=False)
    W1 = nc.declare_dram_parameter("W1", [NE, D, FH], BF16, isOutput=False)
    W2 = nc.declare_dram_parameter("W2", [NE, FH, D], BF16, isOutput=False)
    WC = nc.declare_dram_parameter("WC", [T, NE], F32, isOutput=False)
    B1 = nc.declare_dram_parameter("B1", [FH, NE], F32, isOutput=False)
    OUT = nc.declare_dram_parameter("OUT", [T, D], BF16, isOutput=True)

    with tile.TileContext(nc) as tc:
        with (
            tc.tile_pool(name="const", bufs=1) as const_pool,
            tc.tile_pool(name="hp", bufs=20) as h_pool,
            tc.tile_pool(name="accp", bufs=1) as acc_pool,
            tc.tile_pool(name="tmpp", bufs=2) as tmp_pool,
            tc.tile_pool(name="ph", bufs=4, space="PSUM") as ph_pool,
            tc.tile_pool(name="py", bufs=4, space="PSUM") as py_pool,
        ):
            # ---- persistent staging: everything is SBUF-resident ----
            xT_sb = const_pool.tile([P, DT, T], BF16)
            nc.sync.dma_start(xT_sb[:], xT.rearrange("(dt p) t -> p dt t", p=P))

            wc_sb = const_pool.tile([P, NTT, NE], F32)
            nc.sync.dma_start(wc_sb[:], WC.rearrange("(tt p) e -> p tt e", p=P))

            b1_sb = const_pool.tile([P, FTH, NE], F32)
            nc.sync.dma_start(b1_sb[:], B1.rearrange("(ft p) e -> p ft e", p=P))

            w1_sb = [
                const_pool.tile([P, DT, FH], BF16, name=f"w1r{e}") for e in range(NE)
            ]
            w2_sb = [
                const_pool.tile([P, FTH, D], BF16, name=f"w2r{e}") for e in range(NE)
            ]
            for e in range(NE):
                nc.sync.dma_start(
                    w1_sb[e][:], W1[e].rearrange("(dt p) f -> p dt f", p=P)
                )
                nc.sync.dma_start(
                    w2_sb[e][:], W2[e].rearrange("(ft p) d -> p ft d", p=P)
                )

            acc = acc_pool.tile([P, NTT, D], BF16)

            # ---- expert loop (zero DMA inside) ----
            def expert_loop(_iv):
                for e in range(NE):
                    for cp in range(NTC // 2):
                        tca, tcb = 2 * cp, 2 * cp + 1
                        # GEMM1: hT[f, t] = gelu(W1h^T xT + b1h), f on parts.
                        # Token-chunk pairs share the stationary W1 tile (the
                        # second LDWEIGHTS is elided) and interleave their
                        # accumulations across two PSUM banks.
                        ha = [None] * FTH
                        hb = [None] * FTH
                        for ft in range(FTH):
                            pha = ph_pool.tile([P, TCHUNK], F32, tag="ph", name="pha")
                            phb = ph_pool.tile([P, TCHUNK], F32, tag="ph", name="phb")
                            for dt in range(DT):
                                w = w1_sb[e][:, dt, bass.ts(ft, P)]
                                nc.tensor.matmul(
                                    pha[:],
                                    w,
                                    xT_sb[:, dt, bass.ts(tca, TCHUNK)],
                                    start=(dt == 0),
                                    stop=(dt == DT - 1),
                                )
                                nc.tensor.matmul(
                                    phb[:],
                                    w,
                                    xT_sb[:, dt, bass.ts(tcb, TCHUNK)],
                                    start=(dt == 0),
                                    stop=(dt == DT - 1),
                                )
                            bias = b1_sb[:, ft, e : e + 1]
                            hsa = h_pool.tile([P, TCHUNK], BF16, tag="hsb")
                            nc.scalar.activation(hsa[:], pha[:], GELU, bias=bias)
                            ha[ft] = hsa
                            hsb = h_pool.tile([P, TCHUNK], BF16, tag="hsb")
                            nc.scalar.activation(hsb[:], phb[:], GELU, bias=bias)
                            hb[ft] = hsb

                        # GEMM2 + combine: t on partitions. t-tile pairs
                        # interleave across two PSUM banks.
                        for tc_i, hx in ((tca, ha), (tcb, hb)):
                            for tp in range(TSUB // 2):
                                ta, tb = 2 * tp, 2 * tp + 1
                                pya = py_pool.tile([P, D], F32, tag="py", name="pya")
                                pyb = py_pool.tile([P, D], F32, tag="py", name="pyb")
                                for ft in range(FTH):
                                    w2ap = w2_sb[e][:, ft, :]
                                    nc.tensor.matmul(
                                        pya[:],
                                        hx[ft][:, bass.ts(ta, P)],
                                        w2ap,
                                        start=(ft == 0),
                                        stop=(ft == FTH - 1),
                                    )
                                    nc.tensor.matmul(
                                        pyb[:],
                                        hx[ft][:, bass.ts(tb, P)],
                                        w2ap,
                                        start=(ft == 0),
                                        stop=(ft == FTH - 1),
                                    )
                                for tsub, py in ((ta, pya), (tb, pyb)):
                                    tt = tc_i * TSUB + tsub
                                    wap = wc_sb[:, tt, e : e + 1]
                                    if e == 0:
                                        nc.scalar.mul(acc[:, tt, :], py[:], wap)
                                    else:
                                        tmp = tmp_pool.tile([P, D], F32, tag="tmp")
                                        nc.scalar.mul(tmp[:], py[:], wap)
                                        nc.vector.tensor_add(
                                            acc[:, tt, :], acc[:, tt, :], tmp[:]
                                        )

            if reps == 1:
                expert_loop(0)
            else:
                # staggered_reset: no drain + all-engine barrier on the back
                # edge (PE keeps streaming, HAM stays warm); hint_engines:
                # branch-prefetch the back-edge target (body >> one IRAM
                # block, an unhinted back edge stalls ~3-4us on I$ fetch).
                with tc.For_i(
                    0,
                    reps,
                    1,
                    hint_engines=tuple(mybir.ALL_ENGINES),
                    staggered_reset=True,
                ) as iv:
                    expert_loop(iv)

            # ---- writeback ----
            for tt in range(NTT):
                nc.sync.dma_start(OUT[bass.ts(tt, P), :], acc[:, tt, :])

    _elide_redundant_ldweights(nc)
    _split_excess_waits(nc)
    return nc


_CACHE = {}


def _make_sharded(nc):
    """Wrap a built Bass program in a cached, sharded, jitted executor."""
    import jax
    from jax.sharding import Mesh, PartitionSpec
    from jax.experimental.shard_map import shard_map
    from concourse import bass2jax

    bass2jax.install_neuronx_cc_hook()

    partition_name = nc.partition_id_tensor.name if nc.partition_id_tensor else None
    in_names = []
    out_names = []
    out_avals = []
    zero_outs = []
    for alloc in nc.m.functions[0].allocations:
        if not isinstance(alloc, mybir.MemoryLocationSet):
            continue
        name = alloc.memorylocations[0].name
        if alloc.kind == "ExternalInput":
            if name != partition_name:
                in_names.append(name)
        elif alloc.kind == "ExternalOutput":
            out_names.append(name)
            shape = tuple(alloc.tensor_shape)
            dtype = mybir.dt.np(alloc.dtype)
            out_avals.append(jax.core.ShapedArray(shape, dtype))
            zero_outs.append(np.zeros(shape, dtype))
    n_params = len(in_names)
    n_outs = len(out_avals)
    all_names = in_names + out_names
    if partition_name is not None:
        all_names = all_names + [partition_name]

    def _body(*args):
        operands = list(args)
        if partition_name is not None:
            operands.append(bass2jax.partition_id_tensor())
        outs = bass2jax._bass_exec_p.bind(
            *operands,
            out_avals=tuple(out_avals),
            in_names=tuple(all_names),
            out_names=tuple(out_names),
            lowering_input_output_aliases=(),
            sim_require_finite=True,
            sim_require_nnan=True,
            nc=nc,
        )
        return tuple(outs)

    devices = jax.devices()[:NCORES]
    mesh = Mesh(np.asarray(devices), ("core",))
    in_specs = (PartitionSpec("core"),) * (n_params + n_outs)
    out_specs = (PartitionSpec("core"),) * n_outs
    donate = tuple(range(n_params, n_params + n_outs))
    sharded = jax.jit(
        shard_map(
            _body, mesh=mesh, in_specs=in_specs, out_specs=out_specs, check_rep=False
        ),
        donate_argnums=donate,
        keep_unused=True,
    )

    def runner(in_maps, timeit=False):
        per_core = [[np.asarray(m[nm]) for nm in in_names] for m in in_maps]
        concat_in = [
            np.concatenate([per_core[c][i] for c in range(NCORES)], axis=0)
            for i in range(n_params)
        ]
        concat_zeros = [
            np.zeros((NCORES * z.shape[0], *z.shape[1:]), z.dtype) for z in zero_outs
        ]
        out_arrs = sharded(*concat_in, *concat_zeros)
        return [
            {
                nm: np.asarray(out_arrs[i]).reshape(NCORES, *out_avals[i].shape)[c]
                for i, nm in enumerate(out_names)
            }
            for c in range(NCORES)
        ]

    return runner, (in_names, out_names, out_avals, zero_outs, sharded, mesh)


def _get_runner():
    """Compile once; return a callable(list_of_in_maps) -> list_of_out_maps."""
    if "runner" in _CACHE:
        return _CACHE["runner"]
    nc = _build_program()
    runner, meta = _make_sharded(nc)
    _CACHE["runner"] = runner
    _CACHE["nc"] = nc
    _CACHE["meta"] = meta
    return runner


def _prep_in_maps(
    hidden_states, gating_probs, shared_gate_prob, W1, b1, W2, b2, Ws1, bs1, Ws2, bs2
):
    bf16 = ml_dtypes.bfloat16
    x = np.asarray(hidden_states, np.float32).reshape(NTOK, D)
    g = np.asarray(gating_probs, np.float32).reshape(NTOK, E)
    sg = np.asarray(shared_gate_prob, np.float32).reshape(NTOK, 1)

    # combine weights: routed experts get (1-sg)*g_e, shared expert gets sg
    wc = np.concatenate([(1.0 - sg) * g, sg], axis=1).astype(np.float32)  # [NTOK, 9]

    W1all = np.concatenate(
        [np.asarray(W1, np.float32), np.asarray(Ws1, np.float32)[None]], axis=0
    ).astype(bf16)  # [9, D, F]
    W2all = np.concatenate(
        [np.asarray(W2, np.float32), np.asarray(Ws2, np.float32)[None]], axis=0
    ).astype(bf16)  # [9, F, D]
    B1all = (
        np.concatenate(
            [np.asarray(b1, np.float32), np.asarray(bs1, np.float32)[None]], axis=0
        )
        .T.astype(np.float32)
        .copy()
    )  # [F, 9]
    B2all = np.concatenate(
        [np.asarray(b2, np.float32), np.asarray(bs2, np.float32)[None]], axis=0
    )  # [9, D] — folded on the host: OUT += wc @ B2all (exact fp32)

    in_maps = []
    for c in range(NCORES):
        gidx, h = c // NHALF, c % NHALF
        sl = slice(gidx * T, (gidx + 1) * T)
        fsl = slice(h * FH, (h + 1) * FH)
        in_maps.append(
            {
                "xT": np.ascontiguousarray(x[sl].T).astype(bf16),
                "W1": np.ascontiguousarray(W1all[:, :, fsl]),
                "W2": np.ascontiguousarray(W2all[:, fsl, :]),
                "WC": np.ascontiguousarray(wc[sl]),
                "B1": np.ascontiguousarray(B1all[fsl]),
            }
        )
    return in_maps, wc @ B2all


def kernel(**inputs) -> np.ndarray:
    runner = _get_runner()
    in_maps, bias_img = _prep_in_maps(**inputs)
    results = runner(in_maps)
    # sum the two F-half partials per token group, add the host-folded bias
    out = np.concatenate(
        [
            np.asarray(results[2 * g]["OUT"], np.float32)
            + np.asarray(results[2 * g + 1]["OUT"], np.float32)
            for g in range(NGROUP)
        ],
        axis=0,
    )
    out += bias_img
    return out.reshape(B, S, D)


# revision 3
# speedup vs baseline: 1.9669x; 1.9669x over previous
"""Trainium2 Bass kernel for nn_MoEFFN_86895778333203 — v3.

Dense MoE FFN: B=4, S=2048, D=512, F=2048, E=8 routed experts + 1 shared
expert, gating-weighted combine.

Sharding v2: 2D (token-group x F-half). The 8 cores form a 4x2 grid:
core = 2*group + half. Each core owns 2048 tokens (group) and 1024
d_ff columns (half) of every expert's W1/W2 (+ the shared expert folded
in as expert #8). ALL weights for a core fit in SBUF (18.9 MB bf16), so
after the prologue there is ZERO steady-state DMA — the rep loop is pure
compute. The two halves' partial outputs (contraction over f splits
linearly; gelu is elementwise in f) are summed pairwise on the host,
which also folds in the b2/bs2 bias image.

Per-core device algorithm (all matmuls bf16, fp32 PSUM accumulation):
  hT_e = gelu_tanh(W1h_e^T x^T + b1h_e)      # [Fh,Tc] layout, f on parts
  y_e  = hT_e^T W2h_e                        # [Tc,D], t on partitions
  acc  = sum_e w_e * y_e                     # per-token combine weights
with w_e = (1-sg)*g_e routed, w_8 = sg shared. acc is bf16 in SBUF
(16 KB/partition); host re-sums halves in fp32.

Scheduling notes: GEMM1 processes token-chunk PAIRS sharing the
stationary W1 tile (the second LDWEIGHTS is elided at BIR level) with
the two accumulations interleaved across two PSUM banks; GEMM2
interleaves t-tile pairs across two banks the same way (the baseline's
single-bank runs of 16 were measurably slower).
"""

import numpy as np
import ml_dtypes

import concourse.bass as bass
import concourse.tile as tile
from concourse import mybir

# ---------------------------------------------------------------------------
# Patch TileContext._drain_and_barrier: the stock version attaches one sem
# wait per live logical proc to a single Drain instruction; this walrus
# build caps sync-wait commands per instruction, so split the waits across
# several drains (each observes <=CHUNK procs; same-engine program order
# makes the union equivalent).
# ---------------------------------------------------------------------------
from concourse.vector_clock import ScopedClock, VectorClock

_DRAIN_CHUNK = 4


def _split_drain_and_barrier(self, tick_clock, wait_clock):
    gc = tick_clock.global_clock
    n = len(gc)
    for s in range(0, n, _DRAIN_CHUNK):
        vec = [0] * n
        nonzero = False
        for i in range(s, min(s + _DRAIN_CHUNK, n)):
            vec[i] = gc[i]
            nonzero = nonzero or gc[i] > 0
        if not nonzero:
            continue
        drain_inst = self.nc.sync.drain()
        wait_clock.add_sem_waits(drain_inst.ins, ScopedClock({None: VectorClock(vec)}))
    self.nc.all_engine_barrier()
    assert self.sems is not None
    popped = self.nc._tile_sem_poison_stack.pop()
    assert popped is self._sem_poison
    self.nc.clear_and_free_semaphores(list(self.sems.allocated().values()))
    self.nc.all_engine_barrier()


tile.TileContext._drain_and_barrier = _split_drain_and_barrier


def _split_excess_waits(nc):
    """This walrus build allows at most 1 sync-wait command per instruction
    (2 for EventSemaphore ops). Tile attaches up to ~4. Hoist the excess
    onto standalone EventSemaphore wait instructions inserted immediately
    before the owner in the same block (same engine => program order is
    preserved, semantics identical)."""
    uid = 0
    for fn in nc.m.functions:
        for bb in fn.blocks:
            il = bb.instructions
            i = 0
            while i < len(il):
                inst = il[i]
                si = inst.sync_info
                waits = list(si.on_wait) if si and si.on_wait else []
                cap = 2 if isinstance(inst, mybir.InstEventSemaphore) else 1
                if len(waits) > cap:
                    keep = waits[-cap:]
                    extra = waits[:-cap]
                    new_insts = []
                    for j in range(0, len(extra), 2):
                        uid += 1
                        new_insts.append(
                            mybir.InstEventSemaphore(
                                name=f"bass_splitwait_{uid}",
                                engine=inst.engine,
                                sync_info=mybir.SyncInfo(
                                    on_wait=list(extra[j : j + 2]), on_update=[]
                                ),
                            )
                        )
                    si.on_wait = keep
                    for k, wi in enumerate(new_insts):
                        il.insert(i + k, wi)
                    i += len(new_insts)
                i += 1


def _elide_redundant_ldweights(nc):
    """Drop an InstLdweights that reloads the exact stationary operand the PE
    already holds (same memref/offset/ap/dtype, only matmuls in between, no
    sync attached). walrus's own --enable-ldw-opt pass does this but crashes
    on this toolchain, so do it on the BIR directly: a matmul with no fresh
    LDWEIGHTS keeps using the current foreground weights."""
    for fn in nc.m.functions:
        for bb in fn.blocks:
            il = bb.instructions
            keep = []
            last_key = None
            for inst in il:
                nm = type(inst).__name__
                if nm == "InstLdweights":
                    ap = inst.ins[0]
                    key = (ap.memref, ap.offset, str(ap.ap), str(ap.dtype))
                    si = inst.sync_info
                    clean = not (si and (si.on_wait or si.on_update))
                    if key == last_key and clean:
                        continue  # elide
                    last_key = key
                elif nm != "InstMatmult" and str(inst.engine).endswith("PE"):
                    last_key = None
                keep.append(inst)
            if len(keep) != len(il):
                il[:] = keep


# ---------------------------------------------------------------------------
# Problem shapes (hardcoded per contract)
# ---------------------------------------------------------------------------
B, S, D, F, E = 4, 2048, 512, 2048, 8
NCORES = 8
NGROUP = 4                # token groups
NHALF = 2                 # F halves
NTOK = B * S              # 8192 tokens total
T = NTOK // NGROUP        # 2048 tokens per core (token group)
FH = F // NHALF           # 1024 d_ff columns per core
NE = E + 1                # 8 routed + shared
P = 128
DT = D // P               # 4 k-tiles for GEMM1
FTH = FH // P             # 8 f-tiles per half (GEMM1 m-tiles / GEMM2 k-tiles)
TCHUNK = 512              # GEMM1 rhs free-dim chunk
NTC = T // TCHUNK         # 4 chunks
TSUB = TCHUNK // P        # 4 t-tiles of 128 per chunk
NTT = T // P              # 16 t-tiles per core

BF16 = mybir.dt.bfloat16
F32 = mybir.dt.float32
GELU = mybir.ActivationFunctionType.Gelu_apprx_tanh


def _build_program(reps: int = 1):
    nc = bass.Bass()
    xT = nc.declare_dram_parameter("xT", [D, T], BF16, isOutput=False)
    W1 = nc.declare_dram_parameter("W1", [NE, D, FH], BF16, isOutput=False)
    W2 = nc.declare_dram_parameter("W2", [NE, FH, D], BF16, isOutput=False)
    WC = nc.declare_dram_parameter("WC", [T, NE], F32, isOutput=False)
    B1 = nc.declare_dram_parameter("B1", [FH, NE], F32, isOutput=False)
    OUT = nc.declare_dram_parameter("OUT", [T, D], BF16, isOutput=True)

    with tile.TileContext(nc) as tc:
        with (
            tc.tile_pool(name="const", bufs=1) as const_pool,
            tc.tile_pool(name="hp", bufs=24) as h_pool,
            tc.tile_pool(name="accp", bufs=1) as acc_pool,
            tc.tile_pool(name="ph", bufs=4, space="PSUM") as ph_pool,
            tc.tile_pool(name="py", bufs=4, space="PSUM") as py_pool,
        ):
            # ---- persistent staging: everything is SBUF-resident ----
            xT_sb = const_pool.tile([P, DT, T], BF16)
            nc.sync.dma_start(xT_sb[:], xT.rearrange("(dt p) t -> p dt t", p=P))

            wc_sb = const_pool.tile([P, NTT, NE], F32)
            nc.sync.dma_start(wc_sb[:], WC.rearrange("(tt p) e -> p tt e", p=P))

            b1_sb = const_pool.tile([P, FTH, NE], F32)
            nc.sync.dma_start(b1_sb[:], B1.rearrange("(ft p) e -> p ft e", p=P))

            w1_sb = [
                const_pool.tile([P, DT, FH], BF16, name=f"w1r{e}") for e in range(NE)
            ]
            w2_sb = [
                const_pool.tile([P, FTH, D], BF16, name=f"w2r{e}") for e in range(NE)
            ]
            for e in range(NE):
                nc.sync.dma_start(
                    w1_sb[e][:], W1[e].rearrange("(dt p) f -> p dt f", p=P)
                )
                nc.sync.dma_start(
                    w2_sb[e][:], W2[e].rearrange("(ft p) d -> p ft d", p=P)
                )

            acc = acc_pool.tile([P, NTT, D], BF16)

            # ---- expert loop (zero DMA inside) ----
            def expert_loop(_iv):
                for e in range(NE):
                    for cp in range(NTC // 2):
                        tca, tcb = 2 * cp, 2 * cp + 1
                        # GEMM1: hT[f, t] = gelu(W1h^T xT + b1h), f on parts.
                        # Token-chunk pairs share the stationary W1 tile (the
                        # second LDWEIGHTS is elided) and interleave their
                        # accumulations across two PSUM banks.
                        ha = [None] * FTH
                        hb = [None] * FTH
                        for ft in range(FTH):
                            pha = ph_pool.tile([P, TCHUNK], F32, tag="ph", name="pha")
                            phb = ph_pool.tile([P, TCHUNK], F32, tag="ph", name="phb")
                            for dt in range(DT):
                                w = w1_sb[e][:, dt, bass.ts(ft, P)]
                                nc.tensor.matmul(
                                    pha[:],
                                    w,
                                    xT_sb[:, dt, bass.ts(tca, TCHUNK)],
                                    start=(dt == 0),
                                    stop=(dt == DT - 1),
                                )
                                nc.tensor.matmul(
                                    phb[:],
                                    w,
                                    xT_sb[:, dt, bass.ts(tcb, TCHUNK)],
                                    start=(dt == 0),
                                    stop=(dt == DT - 1),
                                )
                            bias = b1_sb[:, ft, e : e + 1]
                            hsa = h_pool.tile([P, TCHUNK], BF16, tag="hsb")
                            nc.scalar.activation(hsa[:], pha[:], GELU, bias=bias)
                            ha[ft] = hsa
                            hsb = h_pool.tile([P, TCHUNK], BF16, tag="hsb")
                            nc.scalar.activation(hsb[:], phb[:], GELU, bias=bias)
                            hb[ft] = hsb

                        # GEMM2 + combine: t on partitions. t-tile pairs
                        # interleave across two PSUM banks.
                        for tc_i, hx in ((tca, ha), (tcb, hb)):
                            for tp in range(TSUB // 2):
                                ta, tb = 2 * tp, 2 * tp + 1
                                pya = py_pool.tile([P, D], F32, tag="py", name="pya")
                                pyb = py_pool.tile([P, D], F32, tag="py", name="pyb")
                                for ft in range(FTH):
                                    w2ap = w2_sb[e][:, ft, :]
                                    nc.tensor.matmul(
                                        pya[:],
                                        hx[ft][:, bass.ts(ta, P)],
                                        w2ap,
                                        start=(ft == 0),
                                        stop=(ft == FTH - 1),
                                    )
                                    nc.tensor.matmul(
                                        pyb[:],
                                        hx[ft][:, bass.ts(tb, P)],
                                        w2ap,
                                        start=(ft == 0),
                                        stop=(ft == FTH - 1),
                                    )
                                for tsub, py in ((ta, pya), (tb, pyb)):
                                    tt = tc_i * TSUB + tsub
                                    wap = wc_sb[:, tt, e : e + 1]
                                    if e == 0:
                                        nc.vector.tensor_scalar_mul(
                                            acc[:, tt, :], py[:], wap
                                        )
                                    else:
                                        # acc = (py * wc) + acc in one DVE op
                                        nc.vector.scalar_tensor_tensor(
                                            acc[:, tt, :],
                                            py[:],
                                            wap,
                                            acc[:, tt, :],
                                            op0=mybir.AluOpType.mult,
                                            op1=mybir.AluOpType.add,
                                        )

            if reps == 1:
                expert_loop(0)
            else:
                # staggered_reset: no drain + all-engine barrier on the back
                # edge (PE keeps streaming, HAM stays warm); hint_engines:
                # branch-prefetch the back-edge target (body >> one IRAM
                # block, an unhinted back edge stalls ~3-4us on I$ fetch).
                with tc.For_i(
                    0,
                    reps,
                    1,
                    hint_engines=tuple(mybir.ALL_ENGINES),
                    staggered_reset=True,
                ) as iv:
                    expert_loop(iv)

            # ---- writeback ----
            for tt in range(NTT):
                nc.sync.dma_start(OUT[bass.ts(tt, P), :], acc[:, tt, :])

    _elide_redundant_ldweights(nc)
    _split_excess_waits(nc)
    return nc


_CACHE = {}


def _make_sharded(nc):
    """Wrap a built Bass program in a cached, sharded, jitted executor."""
    import jax
    from jax.sharding import Mesh, PartitionSpec
    from jax.experimental.shard_map import shard_map
    from concourse import bass2jax

    bass2jax.install_neuronx_cc_hook()

    partition_name = nc.partition_id_tensor.name if nc.partition_id_tensor else None
    in_names = []
    out_names = []
    out_avals = []
    zero_outs = []
    for alloc in nc.m.functions[0].allocations:
        if not isinstance(alloc, mybir.MemoryLocationSet):
            continue
        name = alloc.memorylocations[0].name
        if alloc.kind == "ExternalInput":
            if name != partition_name:
                in_names.append(name)
        elif alloc.kind == "ExternalOutput":
            out_names.append(name)
            shape = tuple(alloc.tensor_shape)
            dtype = mybir.dt.np(alloc.dtype)
            out_avals.append(jax.core.ShapedArray(shape, dtype))
            zero_outs.append(np.zeros(shape, dtype))
    n_params = len(in_names)
    n_outs = len(out_avals)
    all_names = in_names + out_names
    if partition_name is not None:
        all_names = all_names + [partition_name]

    def _body(*args):
        operands = list(args)
        if partition_name is not None:
            operands.append(bass2jax.partition_id_tensor())
        outs = bass2jax._bass_exec_p.bind(
            *operands,
            out_avals=tuple(out_avals),
            in_names=tuple(all_names),
            out_names=tuple(out_names),
            lowering_input_output_aliases=(),
            sim_require_finite=True,
            sim_require_nnan=True,
            nc=nc,
        )
        return tuple(outs)

    devices = jax.devices()[:NCORES]
    mesh = Mesh(np.asarray(devices), ("core",))
    in_specs = (PartitionSpec("core"),) * (n_params + n_outs)
    out_specs = (PartitionSpec("core"),) * n_outs
    donate = tuple(range(n_params, n_params + n_outs))
    sharded = jax.jit(
        shard_map(
            _body, mesh=mesh, in_specs=in_specs, out_specs=out_specs, check_rep=False
        ),
        donate_argnums=donate,
        keep_unused=True,
    )

    def runner(in_maps, timeit=False):
        per_core = [[np.asarray(m[nm]) for nm in in_names] for m in in_maps]
        concat_in = [
            np.concatenate([per_core[c][i] for c in range(NCORES)], axis=0)
            for i in range(n_params)
        ]
        concat_zeros = [
            np.zeros((NCORES * z.shape[0], *z.shape[1:]), z.dtype) for z in zero_outs
        ]
        out_arrs = sharded(*concat_in, *concat_zeros)
        return [
            {
                nm: np.asarray(out_arrs[i]).reshape(NCORES, *out_avals[i].shape)[c]
                for i, nm in enumerate(out_names)
            }
            for c in range(NCORES)
        ]

    return runner, (in_names, out_names, out_avals, zero_outs, sharded, mesh)


def _get_runner():
    """Compile once; return a callable(list_of_in_maps) -> list_of_out_maps."""
    if "runner" in _CACHE:
        return _CACHE["runner"]
    nc = _build_program()
    runner, meta = _make_sharded(nc)
    _CACHE["runner"] = runner
    _CACHE["nc"] = nc
    _CACHE["meta"] = meta
    return runner


def _prep_in_maps(
    hidden_states, gating_probs, shared_gate_prob, W1, b1, W2, b2, Ws1, bs1, Ws2, bs2
):
    bf16 = ml_dtypes.bfloat16
    x = np.asarray(hidden_states, np.float32).reshape(NTOK, D)
    g = np.asarray(gating_probs, np.float32).reshape(NTOK, E)
    sg = np.asarray(shared_gate_prob, np.float32).reshape(NTOK, 1)

    # combine weights: routed experts get (1-sg)*g_e, shared expert gets sg
    wc = np.concatenate([(1.0 - sg) * g, sg], axis=1).astype(np.float32)  # [NTOK, 9]

    W1all = np.concatenate(
        [np.asarray(W1, np.float32), np.asarray(Ws1, np.float32)[None]], axis=0
    ).astype(bf16)  # [9, D, F]
    W2all = np.concatenate(
        [np.asarray(W2, np.float32), np.asarray(Ws2, np.float32)[None]], axis=0
    ).astype(bf16)  # [9, F, D]
    B1all = (
        np.concatenate(
            [np.asarray(b1, np.float32), np.asarray(bs1, np.float32)[None]], axis=0
        )
        .T.astype(np.float32)
        .copy()
    )  # [F, 9]
    B2all = np.concatenate(
        [np.asarray(b2, np.float32), np.asarray(bs2, np.float32)[None]], axis=0
    )  # [9, D] — folded on the host: OUT += wc @ B2all (exact fp32)

    in_maps = []
    for c in range(NCORES):
        gidx, h = c // NHALF, c % NHALF
        sl = slice(gidx * T, (gidx + 1) * T)
        fsl = slice(h * FH, (h + 1) * FH)
        in_maps.append(
            {
                "xT": np.ascontiguousarray(x[sl].T).astype(bf16),
                "W1": np.ascontiguousarray(W1all[:, :, fsl]),
                "W2": np.ascontiguousarray(W2all[:, fsl, :]),
                "WC": np.ascontiguousarray(wc[sl]),
                "B1": np.ascontiguousarray(B1all[fsl]),
            }
        )
    return in_maps, wc @ B2all


def kernel(**inputs) -> np.ndarray:
    runner = _get_runner()
    in_maps, bias_img = _prep_in_maps(**inputs)
    results = runner(in_maps)
    # sum the two F-half partials per token group, add the host-folded bias
    out = np.concatenate(
        [
            np.asarray(results[2 * g]["OUT"], np.float32)
            + np.asarray(results[2 * g + 1]["OUT"], np.float32)
            for g in range(NGROUP)
        ],
        axis=0,
    )
    out += bias_img
    return out.reshape(B, S, D)
